# revision 16
# baseline (speedup 1.0000x reference)
"""Trainium2 Bass kernel for nn_MultiHeadAttention_66391604462494.

Strategy (tensor-parallel over heads, 8 cores x 2 heads):
  - Host: pre-transpose Q/K/V to [D, B*S] fp16, pre-slice + transpose weights
    per core, fold the 1/sqrt(DK) scale into Wq, and precompute the combined
    multiplicative mask/bias tensor  comb[h, b, tk, tq] = exp(bias[h]).T * (mask[b].T != 0)
    in fp16 (softmax(x) == exp(x)*exp(bias)*mask / rowsum, no max-subtraction
    needed: |scores| <= ~8 so exp never overflows; masked lanes are exactly 0).
  - Device, per core:
      q^T/k^T = (Wq/8)^T.T @ Q^T   [128j, S] per batch   (PE, K=1024 in 8 tiles)
      v^T     = Wv^T.T @ V^T, then PE-transposed to v[t,dk] blocks with an
                appended ones-column (row-sums fall out of the PV matmul free).
      scores^T[tk, tq] = k^T.T @ q^T  (K=64; the two heads run concurrently in
                the PE array via base-partition 0/64 row tiling)
      probs = exp(scores) (ACT, PSUM->SBUF fp16), probs *= comb (split between
                DVE fp16 2x and gpsimd to balance engine load)
      PV with probs as the stationary operand: per 128-wide tq block,
                pv[tq, dk] += probs_tile^T @ v_tile  (PE, N=65 incl. the ones
                column -> rowsum lands on the free dim = per-partition scalar)
      normalize: recip(rowsum) per partition (DVE), tensor_scalar multiply,
                then PE-transpose back to attn^T[dk, tq] for the out proj
      partial^T[do, t] = Wo_c^T.T @ attn  (PE, K=128, fp16 PSUM)  -> DRAM
  - Host: sum the 8 per-core partials, transpose back, add bo.
"""

import os
import sys

import numpy as np

for _p in ("/opt/trn_rl_repo", "/root/.axon_site/_ro/trn_rl_repo"):
    if os.path.isdir(_p) and _p not in sys.path:
        sys.path.insert(0, _p)

import concourse.bass as bass  # noqa: E402
import concourse.mybir as mybir  # noqa: E402
import concourse.tile as tile  # noqa: E402
from concourse import bacc  # noqa: E402
from concourse.bass import ds  # noqa: E402
from concourse.bass_utils import run_bass_kernel_spmd  # noqa: E402
from concourse.masks import make_identity  # noqa: E402

B, S, D, H = 4, 2048, 1024, 16
DK = D // H          # 64
T = B * S            # 8192
NCORES = 8
HPC = H // NCORES    # 2 heads per core
JC = HPC * DK        # 128 = per-core slice of the head dim
NTQ = S // 512       # 4 tq chunks per batch
NTK = S // 128       # 16 tk tiles per batch
NDT = D // 128       # 8 D tiles
F16 = mybir.dt.float16
F32 = mybir.dt.float32
EXP = mybir.ActivationFunctionType.Exp
MULT = mybir.AluOpType.mult

# comb-multiply split: gpsimd (Pool) takes tk tiles [0, POOL_TK), DVE the rest
POOL_TK = 3

# A2A: compute q/k/v projections over this core's T/8 position slice (full D
# output) and exchange slices via an HBM AllToAll, instead of every core
# reading full Q/K/V from HBM (cuts per-core input DMA 50.3MB -> ~19MB).
A2A = True
RG = [[i for i in range(NCORES)]]

TIMING_REPS = 0  # when >0, wrap the body in a For_i repeat loop (bench only)


def _emit(nc, tc, qt, kt, vt, wq, wk, wv, wo, cb, out, stg=None):
    with (
        tc.tile_pool(name="wpool", bufs=1) as wpool,
        tc.tile_pool(name="inpool", bufs=2) as inpool,
        tc.tile_pool(name="qkv", bufs=2) as qkv,
        tc.tile_pool(name="probs", bufs=2) as probsp,
        tc.tile_pool(name="comb", bufs=2) as combp,
        tc.tile_pool(name="attn", bufs=2) as attnp,
        tc.tile_pool(name="attnt", bufs=2) as attntp,
        tc.tile_pool(name="rp", bufs=2) as rpool,
        tc.tile_pool(name="outp", bufs=2) as outp,
        tc.tile_pool(name="pp2", bufs=2, space="PSUM") as pp2,
        tc.tile_pool(name="pvp", bufs=2, space="PSUM") as pvp,
        tc.tile_pool(name="pmix", bufs=2, space="PSUM") as pmix,
    ):
        # ---- constants / weights (one-time) ----
        if A2A:
            pass  # full weights are streamed per od-chunk in phase A
        else:
            wq_sb = wpool.tile([128, NDT, JC], F16, name="wq_sb")
            wk_sb = wpool.tile([128, NDT, JC], F16, name="wk_sb")
            wv_sb = wpool.tile([128, NDT, JC], F16, name="wv_sb")
            nc.sync.dma_start(wq_sb[:], wq.ap().rearrange("(dt p) j -> p dt j", p=128))
            nc.sync.dma_start(wk_sb[:], wk.ap().rearrange("(dt p) j -> p dt j", p=128))
            nc.sync.dma_start(wv_sb[:], wv.ap().rearrange("(dt p) j -> p dt j", p=128))
        wo_sb = wpool.tile([128, NDT, 128], F16, name="wo_sb")
        nc.sync.dma_start(wo_sb[:], wo.ap().rearrange("p (dt o) -> p dt o", dt=NDT))
        ident = wpool.tile([128, 128], F16, name="ident")
        make_identity(nc, ident[:])

        qt_r = qt.ap()
        kt_r = kt.ap()
        vt_r = vt.ap()

        import contextlib
        loop_ctx = (
            tc.For_i(0, TIMING_REPS, 1) if TIMING_REPS > 0 else contextlib.nullcontext()
        )
        with loop_ctx:
          if A2A:
            # ---- phase A: project this core's T/8 position slice (full D out),
            # exchange via HBM AllToAll, gather into resident [128, T] tiles ----
            qT = qkv.tile([128, T], F16, tag="qT", name="qT_a2a", bufs=1)
            kT = qkv.tile([128, T], F16, tag="kT", name="kT_a2a", bufs=1)
            vT = qkv.tile([128, T], F16, tag="vT", name="vT_a2a", bufs=1)
            for t, (xdram, wdram, dst) in enumerate(
                ((qt, wq, qT), (kt, wk, kT), (vt, wv, vT))
            ):
                xin = inpool.tile([128, NDT, 1024], F16, tag="xin", name=f"xa_{t}", bufs=1)
                nc.sync.dma_start(xin[:], xdram.ap())
                sin, sout = stg[t]
                for o in range(NDT):
                    wst = inpool.tile([128, NDT, 128], F16, tag="wst", name=f"wst_{t}_{o}")
                    nc.sync.dma_start(wst[:], wdram.ap()[o])
                    stgt = inpool.tile([128, 1024], F16, tag="stg", name=f"stg_{t}_{o}")
                    for half in range(2):
                        ps = pmix.tile([128, 512], F32, tag="x", name=f"pja_{t}_{o}_{half}")
                        for kt_ in range(NDT):
                            nc.tensor.matmul(
                                ps[:], lhsT=wst[:, kt_, :],
                                rhs=xin[:, kt_, ds(half * 512, 512)],
                                start=(kt_ == 0), stop=(kt_ == NDT - 1),
                            )
                        nc.vector.tensor_copy(stgt[:, ds(half * 512, 512)], ps[:])
                    nc.sync.dma_start(sin.ap()[o], stgt[:])
                nc.gpsimd.collective_compute(
                    "AllToAll", mybir.AluOpType.bypass, RG,
                    ins=[sin.ap()], outs=[sout.ap()],
                )
                for s_ in range(NCORES):
                    nc.sync.dma_start(dst[:, ds(s_ * 1024, 1024)], sout.ap()[s_])

          for b in range(B):
            if not A2A:
                # ---- projections for batch b: q^T, k^T, v^T [128j, 2048t] ----
                qT = qkv.tile([128, S], F16, tag="qT", name=f"qT_{b}")
                kT = qkv.tile([128, S], F16, tag="kT", name=f"kT_{b}")
                vT = qkv.tile([128, S], F16, tag="vT", name=f"vT_{b}")
                for src_r, wsb, dst in ((qt_r, wq_sb, qT), (kt_r, wk_sb, kT), (vt_r, wv_sb, vT)):
                    for tci in range(NTQ):
                        xin = inpool.tile([128, NDT, 512], F16, tag="xin", name=f"xin_{b}_{tci}")
                        nc.sync.dma_start(xin[:], src_r[b * NTQ + tci])
                        ps = pmix.tile([128, 512], F32, tag="x", name=f"proj_{b}_{tci}")
                        for dti in range(NDT):
                            nc.tensor.matmul(
                                ps[:], lhsT=wsb[:, dti, :], rhs=xin[:, dti, :],
                                start=(dti == 0), stop=(dti == NDT - 1),
                            )
                        nc.vector.tensor_copy(dst[:, ds(tci * 512, 512)], ps[:])
            boff = b * S if A2A else 0

            # ---- v^T -> v[t, dk] blocks (+ ones column at dk=64) ----
            v0 = qkv.tile([128, NTK, 65], F16, tag="v0", name=f"v0_{b}")
            v1 = qkv.tile([128, NTK, 65], F16, tag="v1", name=f"v1_{b}")
            # column 64 of v' is all-ones: the PV matmul then yields the probs
            # row-sum on the free dim for free
            nc.gpsimd.memset(v0[:, :, 64:65], 1.0)
            nc.gpsimd.memset(v1[:, :, 64:65], 1.0)
            for blk in range(NTK):
                pst = pmix.tile([128, 128], F16, tag="x", name=f"vtr_{b}_{blk}")
                nc.tensor.transpose(pst[:], vT[:, ds(boff + blk * 128, 128)], ident[:])
                nc.vector.tensor_copy(v0[:, blk, 0:64], pst[:, 0:64])
                nc.vector.tensor_copy(v1[:, blk, 0:64], pst[:, 64:128])

            # ---- attention for batch b ----
            attn16 = attnp.tile([128, S], F16, tag="attn16", name=f"attn_{b}")
            for tqc in range(NTQ):
                probs = [None, None]
                comb = [None, None]
                for h in range(HPC):
                    probs[h] = probsp.tile([128, NTK, 512], F16, tag=f"probs{h}", name=f"pr_{b}_{tqc}_{h}", bufs=1)
                    comb[h] = combp.tile([128, NTK, 512], F16, tag=f"comb{h}", name=f"cb_{b}_{tqc}_{h}", bufs=1)
                    nc.sync.dma_start(comb[h][:], cb.ap()[h, b, tqc])
                # interleave the two heads' K=64 matmuls: adjacent MMs target
                # disjoint PE row groups (base partitions 0 / 64) and run
                # concurrently in the array
                for tkp in range(NTK // 2):
                    for h in range(HPC):
                        ps2 = pp2.tile([128, 1024], F32, tag="s2", name=f"sc_{b}_{tqc}_{h}_{tkp}")
                        for half in range(2):
                            tk = tkp * 2 + half
                            nc.tensor.matmul(
                                ps2[:, ds(half * 512, 512)],
                                lhsT=kT[ds(h * 64, 64), ds(boff + tk * 128, 128)],
                                rhs=qT[ds(h * 64, 64), ds(boff + tqc * 512, 512)],
                                start=True, stop=True,
                            )
                        nc.scalar.activation(probs[h][:, ds(tkp * 2, 2), :], ps2[:], EXP)
                for h in range(HPC):
                    # comb multiply split across Pool (SBUF-only engine) + DVE
                    nc.gpsimd.tensor_tensor(
                        probs[h][:, 0:POOL_TK, :], probs[h][:, 0:POOL_TK, :],
                        comb[h][:, 0:POOL_TK, :], op=MULT,
                    )
                    nc.vector.tensor_tensor(
                        probs[h][:, POOL_TK:NTK, :], probs[h][:, POOL_TK:NTK, :],
                        comb[h][:, POOL_TK:NTK, :], op=MULT,
                    )
                # PV with probs stationary: per 128-wide tq block, both heads
                # into one PSUM tile; col 64/129 hold the rowsums
                for jb in range(4):
                    pv = pvp.tile([128, 130], F32, tag="pv", name=f"pv_{b}_{tqc}_{jb}")
                    for h in range(HPC):
                        vh = v0 if h == 0 else v1
                        for tk in range(NTK):
                            nc.tensor.matmul(
                                pv[:, ds(h * 65, 65)],
                                lhsT=probs[h][:, tk, ds(jb * 128, 128)],
                                rhs=vh[:, tk, :],
                                start=(tk == 0), stop=(tk == NTK - 1),
                            )
                    r = rpool.tile([128, 2], F32, tag="r", name=f"r_{b}_{tqc}_{jb}")
                    nc.vector.reciprocal_approx_fast(r[:, 0:1], pv[:, 64:65])
                    nc.vector.reciprocal_approx_fast(r[:, 1:2], pv[:, 129:130])
                    at = attntp.tile([128, 128], F16, tag="at", name=f"at_{b}_{tqc}_{jb}")
                    nc.vector.tensor_scalar_mul(at[:, 0:64], pv[:, 0:64], r[:, 0:1])
                    nc.vector.tensor_scalar_mul(at[:, 64:128], pv[:, 65:129], r[:, 1:2])
                    # transpose [tq, dk2] -> [dk2, tq] for the output proj
                    pt = pmix.tile([128, 128], F16, tag="x", name=f"atr_{b}_{tqc}_{jb}")
                    nc.tensor.transpose(pt[:], at[:], ident[:])
                    nc.vector.tensor_copy(
                        attn16[:, ds((tqc * 4 + jb) * 128, 128)], pt[:]
                    )

            # ---- output projection for batch b (fp16 PSUM: single-shot MMs) ----
            for tqc in range(NTQ):
                for dp in range(NDT // 2):
                    po = pp2.tile([128, 1024], F32, tag="s2", name=f"op_{b}_{tqc}_{dp}")
                    for half in range(2):
                        nc.tensor.matmul(
                            po[:, ds(half * 512, 512)],
                            lhsT=wo_sb[:, dp * 2 + half, :],
                            rhs=attn16[:, ds(tqc * 512, 512)],
                            start=True, stop=True,
                        )
                    ost = outp.tile([128, 1024], F16, tag="ost", name=f"ost_{b}_{tqc}_{dp}")
                    nc.vector.tensor_copy(ost[:], po[:])
                    nc.sync.dma_start(out.ap()[b, tqc, dp], ost[:])


_NC_CACHE = None


def _build_bass():
    global _NC_CACHE
    if _NC_CACHE is not None:
        return _NC_CACHE
    nc = bacc.Bacc("TRN2", target_bir_lowering=False, debug=False, num_devices=NCORES)
    if A2A:
        # per-core position slice of X^T: [p, kt, pos]
        qt = nc.dram_tensor("qt", [128, NDT, 1024], F16, kind="ExternalInput")
        kt = nc.dram_tensor("kt", [128, NDT, 1024], F16, kind="ExternalInput")
        vt = nc.dram_tensor("vt", [128, NDT, 1024], F16, kind="ExternalInput")
        # full transposed weights pre-tiled [o, p, kt, j]
        wq = nc.dram_tensor("wq", [NDT, 128, NDT, 128], F16, kind="ExternalInput")
        wk = nc.dram_tensor("wk", [NDT, 128, NDT, 128], F16, kind="ExternalInput")
        wv = nc.dram_tensor("wv", [NDT, 128, NDT, 128], F16, kind="ExternalInput")
    else:
        # pre-tiled on host: [b*tci, p, dt, t] so every DMA is one contiguous 1 MB read
        qt = nc.dram_tensor("qt", [B * NTQ, 128, NDT, 512], F16, kind="ExternalInput")
        kt = nc.dram_tensor("kt", [B * NTQ, 128, NDT, 512], F16, kind="ExternalInput")
        vt = nc.dram_tensor("vt", [B * NTQ, 128, NDT, 512], F16, kind="ExternalInput")
        wq = nc.dram_tensor("wq", [D, JC], F16, kind="ExternalInput")
        wk = nc.dram_tensor("wk", [D, JC], F16, kind="ExternalInput")
        wv = nc.dram_tensor("wv", [D, JC], F16, kind="ExternalInput")
    wo = nc.dram_tensor("wo", [JC, D], F16, kind="ExternalInput")
    # pre-tiled on host: [h, b, tqc, tki, tko, tq] — contiguous 2 MB per DMA
    cb = nc.dram_tensor("cb", [HPC, B, NTQ, 128, NTK, 512], F16, kind="ExternalInput")
    out = nc.dram_tensor("out", [B, NTQ, NDT // 2, 128, 1024], F16, kind="ExternalOutput")
    stg = None
    if A2A:
        stg = []
        for t in range(3):
            sin = nc.dram_tensor(f"a2a_in_{t}", [NCORES, 128, 1024], F16, kind="Internal")
            sout = nc.dram_tensor(f"a2a_out_{t}", [NCORES, 128, 1024], F16, kind="Internal")
            stg.append((sin, sout))
    with tile.TileContext(nc) as tc:
        _emit(nc, tc, qt, kt, vt, wq, wk, wv, wo, cb, out, stg=stg)
    nc.finalize()
    _NC_CACHE = nc
    return nc


def _tile_xT(X):
    # [T, D] -> X^T tiled as [b*tci, p, dt, t] (contiguous per [128, NDT, 512] tile)
    xt = X.reshape(T, D).T.astype(np.float16)          # [D, T] = [dt*128+p, ...]
    xt = xt.reshape(NDT, 128, B * NTQ, 512)            # [dt, p, b*tci, t]
    return np.ascontiguousarray(np.transpose(xt, (2, 1, 0, 3)))


def _slice_xT(X, c):
    # [T, D] -> X^T[:, c*1024:(c+1)*1024] as [p, kt, pos]
    xt = X.reshape(T, D).T.astype(np.float16)
    sl = xt[:, c * 1024:(c + 1) * 1024].reshape(NDT, 128, 1024)
    return np.ascontiguousarray(np.transpose(sl, (1, 0, 2)))


def _prepare_in_maps(Q, K, V, mask, attn_bias, Wq, Wk, Wv):
    f16 = np.float16
    if A2A:
        def _tile_w(wT):
            # [1024 in, 1024 out] -> [o, p, kt, j]
            arr = wT.astype(f16).reshape(NDT, 128, NDT, 128)
            return np.ascontiguousarray(np.transpose(arr, (2, 1, 0, 3)))

        wq_f = _tile_w(Wq.T / np.sqrt(DK))
        wk_f = _tile_w(Wk.T)
        wv_f = _tile_w(Wv.T)
    else:
        qt = _tile_xT(Q)
        kt = _tile_xT(K)
        vt = _tile_xT(V)
    # mask transposed per batch, as bool [B, Sk, Sq]
    mT = (np.transpose(mask[:, 0], (0, 2, 1)) != 0)
    in_maps = []
    for c in range(NCORES):
        sl = slice(c * JC, (c + 1) * JC)
        if A2A:
            qt = _slice_xT(Q, c)
            kt = _slice_xT(K, c)
            vt = _slice_xT(V, c)
            wq_c, wk_c, wv_c = wq_f, wk_f, wv_f
        else:
            wq_c = np.ascontiguousarray((Wq[sl].T / np.sqrt(DK))).astype(f16)
            wk_c = np.ascontiguousarray(Wk[sl].T).astype(f16)
            wv_c = np.ascontiguousarray(Wv[sl].T).astype(f16)
        wo_c = np.ascontiguousarray(_WO_GLOBAL[:, sl].T).astype(f16)
        comb = np.empty((HPC, B, NTQ, 128, NTK, 512), f16)
        for hh in range(HPC):
            ebT = np.exp(attn_bias[0, c * HPC + hh].astype(np.float64)).T.astype(f16)
            for b in range(B):
                cbb = np.where(mT[b], ebT, f16(0))     # [tk, tq]
                cbb = cbb.reshape(NTK, 128, NTQ, 512)  # [tko, tki, tqc, tq]
                comb[hh, b] = np.transpose(cbb, (2, 1, 0, 3))
        in_maps.append({
            "qt": qt, "kt": kt, "vt": vt,
            "wq": wq_c, "wk": wk_c, "wv": wv_c, "wo": wo_c,
            "cb": comb,
        })
    return in_maps


_WO_GLOBAL = None


def _postprocess(results, bo):
    acc = np.zeros((D, T), np.float32)
    for r in results:
        arr = r["out"].reshape(B, NTQ, NDT // 2, 128, 2, 512)
        acc += np.transpose(arr, (2, 4, 3, 0, 1, 5)).reshape(D, T)
    out = acc.T + bo[None, :].astype(np.float32)
    return out.reshape(B, S, D).astype(np.float32)


def _run(inputs, trace=False):
    global _WO_GLOBAL
    _WO_GLOBAL = np.asarray(inputs["Wo"], np.float32)
    nc = _build_bass()
    in_maps = _prepare_in_maps(
        np.asarray(inputs["Q"], np.float32), np.asarray(inputs["K"], np.float32),
        np.asarray(inputs["V"], np.float32), np.asarray(inputs["mask"]),
        np.asarray(inputs["attn_bias"], np.float32), np.asarray(inputs["Wq"], np.float32),
        np.asarray(inputs["Wk"], np.float32), np.asarray(inputs["Wv"], np.float32),
    )
    res = run_bass_kernel_spmd(nc, in_maps, core_ids=list(range(NCORES)), trace=trace)
    out = _postprocess(res.results, np.asarray(inputs["bo"], np.float32))
    return out, res


def kernel(**inputs):
    out, _ = _run(inputs, trace=False)
    return out
